# revision 1
# baseline (speedup 1.0000x reference)
"""Trainium2 Bass kernel for the NonIsotropic vMF head.

Contract: kernel(**inputs) takes FULL unsharded inputs (as produced by
setup_inputs()) and returns the FULL [S=8, B=64, C=1000] float32 output.

Strategy
--------
The [S,B,C,D] intermediate collapses algebraically:
    cos[s,b,c]  = (X @ (kap*scm)^T) * rsqrt(X^2 @ (kap^2)^T)   (X = samples [S*B, D])
    out[s,b,c]  = const[c] + cos[s,b,c]            (scm_norm folded into the numerator)
The RNG draws (beta/uniform/normal, key 42) are input-independent; they are
generated host-side with the exact same jax.random calls the reference makes
(on CPU — the reference cannot run on the axon backend) and shipped to the
device as constants.  All input-dependent compute (MLP -> kappa, rejection
accept/select, Householder, class stats, big matmuls) runs on device.

Sharding: classes C=1000 are split 125-per-core over 8 cores (sampling is
replicated — every core needs all samples).  Outputs are concatenated on the
class axis on the host.
"""

import numpy as np

S, B, D, K, C, H = 8, 64, 128, 32, 1000, 256
NCORES = 8
CLOC = C // NCORES            # 125 classes per core
SB = S * B                    # 512
M1 = float(D - 1)             # 127.0
LN127 = float(np.log(M1))
LN2PI = float(np.log(2.0 * np.pi))

_cache = {}


def _host_constants():
    """RNG constants of the reference sampler (input-independent, key 42)."""
    if "rng" in _cache:
        return _cache["rng"]
    import jax
    import jax.numpy as jnp

    cpu = jax.devices("cpu")[0]
    with jax.default_device(cpu):
        key = jax.random.key(42)
        k_eps, k_u, k_v = jax.random.split(key, 3)
        alpha = M1 / 2.0
        eps = np.asarray(jax.random.beta(k_eps, alpha, alpha, (K, S, B)), np.float32)
        u = jax.random.uniform(k_u, (K, S, B), jnp.float32, minval=1e-7, maxval=1.0)
        logu = np.asarray(jnp.log(u), np.float32)
        vraw = jax.random.normal(k_v, (S, B, D - 1), jnp.float32)
        vn = np.asarray(
            vraw / jnp.maximum(jnp.linalg.norm(vraw, axis=-1, keepdims=True), 1e-12),
            np.float32,
        )
    # device layouts: [b, s-major, r/d-inner]
    eps_b = np.ascontiguousarray(np.transpose(eps, (2, 1, 0)).reshape(B, S * K))
    logu_b = np.ascontiguousarray(np.transpose(logu, (2, 1, 0)).reshape(B, S * K))
    v_b = np.ascontiguousarray(np.transpose(vn, (1, 0, 2)).reshape(B, S * (D - 1)))
    ident = np.eye(128, dtype=np.float32)
    _cache["rng"] = (eps_b, logu_b, v_b, ident)
    return _cache["rng"]


def build_nc(stage=99):
    """Build the per-core Bass program (SPMD: same program, per-core class shard).

    stage < 99 builds a prefix of the pipeline (debug bisection); the output
    tensor is still declared but only partially written.
    """
    import concourse.bass as bass
    import concourse.mybir as mybir
    from concourse import bacc, tile

    fp = mybir.dt.float32
    Alu = mybir.AluOpType
    Act = mybir.ActivationFunctionType

    nc = bacc.Bacc(None)

    # ---- DRAM parameters ----
    d_feat = nc.declare_dram_parameter("features", [B, D], fp, isOutput=False)
    d_W0 = nc.declare_dram_parameter("W0", [H, D], fp, isOutput=False)
    d_b0 = nc.declare_dram_parameter("b0", [H], fp, isOutput=False)
    d_W1 = nc.declare_dram_parameter("W1", [H, H], fp, isOutput=False)
    d_b1 = nc.declare_dram_parameter("b1", [H], fp, isOutput=False)
    d_W2 = nc.declare_dram_parameter("W2", [1, H], fp, isOutput=False)
    d_b2 = nc.declare_dram_parameter("b2", [1], fp, isOutput=False)
    d_wmu = nc.declare_dram_parameter("wmu", [CLOC, D], fp, isOutput=False)
    d_wk = nc.declare_dram_parameter("wk", [CLOC, D], fp, isOutput=False)
    d_eps = nc.declare_dram_parameter("eps", [B, S * K], fp, isOutput=False)
    d_logu = nc.declare_dram_parameter("logu", [B, S * K], fp, isOutput=False)
    d_v = nc.declare_dram_parameter("vt", [B, S * (D - 1)], fp, isOutput=False)
    d_id = nc.declare_dram_parameter("ident", [128, 128], fp, isOutput=False)
    d_out = nc.declare_dram_parameter("out", [SB, CLOC], fp, isOutput=True)

    def _emit(tc):
        with (
            tc.tile_pool(name="w", bufs=1) as wp,          # weights / persistent
            tc.tile_pool(name="s", bufs=1) as sp,          # stage tensors
            tc.tile_pool(name="scr", bufs=4) as scrp,      # scratch
            tc.tile_pool(name="ptp", bufs=2, space="PSUM") as ptp,    # transposes
            tc.tile_pool(name="pmm", bufs=2, space="PSUM") as pmm,    # mlp matmuls
            tc.tile_pool(name="pn", bufs=2, space="PSUM") as pnp,     # num matmuls
            tc.tile_pool(name="pd", bufs=2, space="PSUM") as pdp,     # den matmuls
        ):
            # ================= loads =================
            ident = wp.tile([128, 128], fp)
            nc.sync.dma_start(ident[:], d_id[:])
            feat = wp.tile([B, D], fp)
            nc.sync.dma_start(feat[:], d_feat[:])
            eps = wp.tile([B, S * K], fp)
            nc.sync.dma_start(eps[:], d_eps[:])
            logu = wp.tile([B, S * K], fp)
            nc.sync.dma_start(logu[:], d_logu[:])
            vt = wp.tile([B, S * (D - 1)], fp)
            nc.sync.dma_start(vt[:], d_v[:])
            w0blk = [wp.tile([128, D], fp, name=f"w0blk{j}") for j in range(2)]
            for j in range(2):
                nc.sync.dma_start(w0blk[j][:], d_W0[j * 128:(j + 1) * 128, :])
            w1blk = [[wp.tile([128, 128], fp, name=f"w1blk{j}{i}") for i in range(2)]
                     for j in range(2)]
            for j in range(2):
                for i in range(2):
                    nc.sync.dma_start(
                        w1blk[j][i][:],
                        d_W1[j * 128:(j + 1) * 128, i * 128:(i + 1) * 128])
            b0t = wp.tile([128, 2], fp)    # column j = bias chunk j
            b1t = wp.tile([128, 2], fp)
            w2t = wp.tile([128, 2], fp)    # column k = W2[0, k*128:(k+1)*128]
            for j in range(2):
                nc.sync.dma_start(
                    b0t[:, j:j + 1],
                    d_b0[j * 128:(j + 1) * 128].rearrange("(p o) -> p o", o=1))
                nc.sync.dma_start(
                    b1t[:, j:j + 1],
                    d_b1[j * 128:(j + 1) * 128].rearrange("(p o) -> p o", o=1))
                nc.sync.dma_start(
                    w2t[:, j:j + 1],
                    d_W2[0:1, j * 128:(j + 1) * 128].rearrange("a (p o) -> (a p) o", o=1))
            b2t = wp.tile([1, 1], fp)
            nc.sync.dma_start(b2t[:], d_b2[0:1].rearrange("(a o) -> a o", o=1))
            wmu = wp.tile([CLOC, D], fp)
            nc.sync.dma_start(wmu[:], d_wmu[:])
            wk = wp.tile([CLOC, D], fp)
            nc.sync.dma_start(wk[:], d_wk[:])

            ones = wp.tile([1, 128], fp)
            nc.gpsimd.memset(ones[:], 1.0)

            if stage < 20:
                return
            # ================= MLP -> kappa =================
            # xT [D, B]
            ps = ptp.tile([128, B], fp, tag="tp")
            nc.tensor.transpose(ps[:], feat[:], ident[0:B, 0:B])
            xT = sp.tile([D, B], fp)
            nc.scalar.copy(xT[:], ps[:])

            # W0T [D, H] (two PE transposes)
            w0T = sp.tile([D, H], fp)
            for j in range(2):
                ps = ptp.tile([128, 128], fp, tag="tp")
                nc.tensor.transpose(ps[:], w0blk[j][:], ident[:, :])
                nc.vector.tensor_copy(w0T[:, j * 128:(j + 1) * 128], ps[:])

            # W1T blocks [k=i, m=j] = transpose(W1[j, i])
            w1T = [[sp.tile([128, 128], fp, name=f"w1T{i}{j}") for j in range(2)]
                   for i in range(2)]
            for i in range(2):
                for j in range(2):
                    ps = ptp.tile([128, 128], fp, tag="tp")
                    nc.tensor.transpose(ps[:], w1blk[j][i][:], ident[:, :])
                    nc.vector.tensor_copy(w1T[i][j][:], ps[:])

            # h0T = relu(W0T.T @ xT + b0)  -> two [128, B] tiles
            h0r = [sp.tile([128, B], fp, name=f"h0r{j}") for j in range(2)]
            for j in range(2):
                pm = pmm.tile([128, B], fp, tag="mm")
                nc.tensor.matmul(pm[:], w0T[:, j * 128:(j + 1) * 128], xT[:],
                                 start=True, stop=True)
                nc.scalar.activation(h0r[j][:], pm[:], Act.Relu,
                                     bias=b0t[:, j:j + 1], scale=1.0)

            # h1T = relu(W1T.T @ h0 + b1)
            h1r = [sp.tile([128, B], fp, name=f"h1r{j}") for j in range(2)]
            for j in range(2):
                pm = pmm.tile([128, B], fp, tag="mm")
                nc.tensor.matmul(pm[:], w1T[0][j][:], h0r[0][:], start=True, stop=False)
                nc.tensor.matmul(pm[:], w1T[1][j][:], h0r[1][:], start=False, stop=True)
                nc.scalar.activation(h1r[j][:], pm[:], Act.Relu,
                                     bias=b1t[:, j:j + 1], scale=1.0)

            # h2 [B,1] = h1 @ W2.T + b2 ; kappa = softplus(h2) + 1e-6
            pm = pmm.tile([B, 1], fp, tag="mm")
            nc.tensor.matmul(pm[:], h1r[0][:], w2t[:, 0:1], start=True, stop=False)
            nc.tensor.matmul(pm[:], h1r[1][:], w2t[:, 1:2], start=False, stop=False)
            nc.tensor.matmul(pm[:], ones[0:1, 0:B], b2t[:], start=False, stop=True)
            eh2 = sp.tile([B, 1], fp)
            nc.scalar.activation(eh2[:], pm[:], Act.Exp)           # e^h2
            kap_b = sp.tile([B, 1], fp)
            nc.scalar.activation(kap_b[:], eh2[:], Act.Ln, bias=1.0, scale=1.0)  # ln(1+e^h2)
            nc.vector.tensor_scalar_add(kap_b[:], kap_b[:], 1e-6)

            if stage < 30:
                return
            # ================= sampler scalars (per-b [B,1]) =================
            k2 = scrp.tile([B, 1], fp, tag="sc")
            nc.vector.tensor_mul(k2[:], kap_b[:], kap_b[:])
            nc.vector.tensor_scalar(k2[:], k2[:], 4.0, M1 * M1, Alu.mult, Alu.add)
            sq = sp.tile([B, 1], fp)
            nc.scalar.activation(sq[:], k2[:], Act.Sqrt)
            b_ = sp.tile([B, 1], fp)
            nc.vector.scalar_tensor_tensor(b_[:], kap_b[:], -2.0, sq[:],
                                           op0=Alu.mult, op1=Alu.add)
            nc.vector.tensor_scalar_mul(b_[:], b_[:], 1.0 / M1)
            a_ = sp.tile([B, 1], fp)
            nc.vector.scalar_tensor_tensor(a_[:], kap_b[:], 2.0, sq[:],
                                           op0=Alu.mult, op1=Alu.add)
            nc.vector.tensor_scalar(a_[:], a_[:], M1, 0.25, Alu.add, Alu.mult)
            ab = sp.tile([B, 1], fp)
            nc.vector.tensor_mul(ab[:], a_[:], b_[:])
            opb = scrp.tile([B, 1], fp, tag="sc")
            nc.vector.tensor_scalar_add(opb[:], b_[:], 1.0)
            r1pb = scrp.tile([B, 1], fp, tag="sc")
            nc.vector.reciprocal(r1pb[:], opb[:])
            d_ = sp.tile([B, 1], fp)
            nc.vector.scalar_tensor_tensor(d_[:], ab[:], 4.0, r1pb[:],
                                           op0=Alu.mult, op1=Alu.mult)
            nc.vector.tensor_scalar_add(d_[:], d_[:], -M1 * LN127)
            l2ab = sp.tile([B, 1], fp)
            nc.scalar.activation(l2ab[:], ab[:], Act.Ln, scale=2.0)
            E = sp.tile([B, 1], fp)
            nc.vector.scalar_tensor_tensor(E[:], l2ab[:], M1, d_[:],
                                           op0=Alu.mult, op1=Alu.add)
            p2ab = sp.tile([B, 1], fp)
            nc.vector.tensor_scalar_mul(p2ab[:], ab[:], 2.0)
            ncm = sp.tile([B, 1], fp)     # -(1-b) = b-1
            nc.vector.tensor_scalar_add(ncm[:], b_[:], -1.0)
            ncp = sp.tile([B, 1], fp)     # -(1+b)
            nc.vector.tensor_scalar(ncp[:], b_[:], -1.0, -1.0, Alu.mult, Alu.add)

            if stage < 40:
                return
            # ================= accept + first-accept select [B, S*K] =================
            SK = S * K
            x_ = sp.tile([B, SK], fp)     # x = (b-1)*eps  (= denom - 1)
            nc.vector.tensor_scalar_mul(x_[:], eps[:], ncm[:])
            denom = sp.tile([B, SK], fp)
            nc.vector.tensor_scalar_add(denom[:], x_[:], 1.0)
            # logden = log1p(x) via 5-term Horner (|x| < ~0.06 here)
            acc = sp.tile([B, SK], fp)
            nc.vector.tensor_scalar(acc[:], x_[:], 0.0, 0.2, Alu.mult, Alu.add)
            for c_ in (-0.25, 1.0 / 3.0, -0.5, 1.0):
                nc.vector.scalar_tensor_tensor(acc[:], acc[:], c_, x_[:],
                                               op0=Alu.add, op1=Alu.mult)
            rec = sp.tile([B, SK], fp)
            nc.vector.reciprocal(rec[:], denom[:])
            s1 = sp.tile([B, SK], fp)     # E - 127*logden
            nc.scalar.activation(s1[:], acc[:], Act.Identity, bias=E[:], scale=-M1)
            s2 = sp.tile([B, SK], fp)     # 2ab*rec + logu
            nc.vector.scalar_tensor_tensor(s2[:], rec[:], p2ab[:], logu[:],
                                           op0=Alu.mult, op1=Alu.add)
            A = sp.tile([B, SK], fp)      # accept = (s1 >= s2)
            nc.vector.scalar_tensor_tensor(A[:], s1[:], 0.0, s2[:],
                                           op0=Alu.bypass, op1=Alu.is_ge)
            # reset-mask: 0 at r==0 columns, 1 elsewhere
            rmask = sp.tile([B, SK], fp)
            nc.gpsimd.memset(rmask[:], 1.0)
            rmask_v = rmask.rearrange("p (s r) -> p s r", r=K)
            nc.gpsimd.memset(rmask_v[:, :, 0:1], 0.0)
            # prefix-max with per-group reset: P = max(rmask*P_prev, A)
            P = sp.tile([B, SK], fp)
            nc.vector.tensor_tensor_scan(P[:], rmask[:], A[:], 0.0,
                                         op0=Alu.mult, op1=Alu.max)
            Pm1 = sp.tile([B, SK], fp)
            nc.vector.tensor_copy(Pm1[:, 1:SK], P[:, 0:SK - 1])
            Pm1_v = Pm1.rearrange("p (s r) -> p s r", r=K)
            nc.gpsimd.memset(Pm1_v[:, :, 0:1], 0.0)
            first = sp.tile([B, SK], fp)
            nc.vector.tensor_sub(first[:], P[:], Pm1[:])
            prod = sp.tile([B, SK], fp)
            nc.vector.tensor_mul(prod[:], eps[:], first[:])
            esel = sp.tile([B, S], fp)
            nc.vector.tensor_reduce(esel[:],
                                    prod.rearrange("p (s r) -> p s r", r=K),
                                    axis=mybir.AxisListType.X, op=Alu.add)
            # all-reject fallback -> round 0 (argmax semantics)
            fb = scrp.tile([B, S], fp, tag="sc8")
            nc.vector.scalar_tensor_tensor(fb[:], P[:, K - 1::K], 1.0, eps[:, 0::K],
                                           op0=Alu.subtract, op1=Alu.mult)
            nc.vector.tensor_sub(esel[:], esel[:], fb[:])

            if stage < 50:
                return
            # ================= w, z, Householder =================
            n1 = scrp.tile([B, S], fp, tag="sc8")
            nc.vector.tensor_scalar_mul(n1[:], esel[:], ncp[:])
            nc.vector.tensor_scalar_add(n1[:], n1[:], 1.0)
            d1 = scrp.tile([B, S], fp, tag="sc8")
            nc.vector.tensor_scalar_mul(d1[:], esel[:], ncm[:])
            nc.vector.tensor_scalar_add(d1[:], d1[:], 1.0)
            rd1 = scrp.tile([B, S], fp, tag="sc8")
            nc.vector.reciprocal(rd1[:], d1[:])
            w_ = sp.tile([B, S], fp)
            nc.vector.tensor_mul(w_[:], n1[:], rd1[:])
            w2_ = scrp.tile([B, S], fp, tag="sc8")
            nc.vector.tensor_mul(w2_[:], w_[:], w_[:])
            cw = scrp.tile([B, S], fp, tag="sc8")
            nc.scalar.activation(cw[:], w2_[:], Act.Relu, bias=1.0, scale=-1.0)
            sm = sp.tile([B, S], fp)
            nc.scalar.activation(sm[:], cw[:], Act.Sqrt)

            if stage < 52:
                return
            # normalize(features) -> mu_n ; uh = normalize(e1 - mu_n)
            fsq = scrp.tile([B, D], fp, tag="scBD")
            ssf = scrp.tile([B, 1], fp, tag="sc")
            nc.scalar.activation(fsq[:], feat[:], Act.Square, accum_out=ssf[:])
            nf = scrp.tile([B, 1], fp, tag="sc")
            nc.scalar.activation(nf[:], ssf[:], Act.Sqrt)
            nc.vector.tensor_scalar_max(nf[:], nf[:], 1e-12)
            rnf = scrp.tile([B, 1], fp, tag="sc")
            nc.vector.reciprocal(rnf[:], nf[:])
            mun = sp.tile([B, D], fp)
            nc.vector.tensor_scalar_mul(mun[:], feat[:], rnf[:])
            em = sp.tile([B, D], fp)
            nc.vector.tensor_scalar_mul(em[:], mun[:], -1.0)
            nc.vector.tensor_scalar_add(em[:, 0:1], em[:, 0:1], 1.0)
            esq = scrp.tile([B, D], fp, tag="scBD")
            sse = scrp.tile([B, 1], fp, tag="sc")
            nc.scalar.activation(esq[:], em[:], Act.Square, accum_out=sse[:])
            ne = scrp.tile([B, 1], fp, tag="sc")
            nc.scalar.activation(ne[:], sse[:], Act.Sqrt)
            nc.vector.tensor_scalar_max(ne[:], ne[:], 1e-12)
            rne = scrp.tile([B, 1], fp, tag="sc")
            nc.vector.reciprocal(rne[:], ne[:])
            uh = sp.tile([B, D], fp)
            nc.vector.tensor_scalar_mul(uh[:], em[:], rne[:])

            if stage < 54:
                return
            # z [B, S*D]: col d=0 is w, cols 1.. are sm * v
            z = sp.tile([B, S * D], fp)
            z_v = z.rearrange("p (s d) -> p s d", d=D)
            vt_v = vt.rearrange("p (s d) -> p s d", d=D - 1)
            nc.vector.tensor_copy(z_v[:, :, 0:1], w_.rearrange("p (s o) -> p s o", o=1))
            for s in range(S):
                nc.vector.tensor_scalar_mul(z_v[:, s, 1:D], vt_v[:, s, :],
                                            sm[:, s:s + 1])
            if stage < 56:
                return
            # dp[b,s] = <uh, z_s> ; samples = z - 2 dp uh
            dp = sp.tile([B, S], fp)
            for s in range(S):
                scr = scrp.tile([B, D], fp, tag="scBD")
                nc.vector.tensor_mul(scr[:], z[:, s * D:(s + 1) * D], uh[:])
                nc.vector.tensor_reduce(dp[:, s:s + 1], scr[:],
                                        axis=mybir.AxisListType.X, op=Alu.add)
            m2dp = sp.tile([B, S], fp)
            nc.vector.tensor_scalar_mul(m2dp[:], dp[:], -2.0)
            smp = sp.tile([B, S * D], fp)
            for s in range(S):
                nc.vector.scalar_tensor_tensor(
                    smp[:, s * D:(s + 1) * D], uh[:], m2dp[:, s:s + 1],
                    z[:, s * D:(s + 1) * D], op0=Alu.mult, op1=Alu.add)

            if stage < 60:
                return
            # transpose to [D, SB]; squares
            sampT = sp.tile([D, SB], fp)
            for s in range(S):
                ps = ptp.tile([128, B], fp, tag="tp")
                nc.tensor.transpose(ps[:], smp[:, s * D:(s + 1) * D], ident[0:B, 0:B])
                if s % 2 == 0:
                    nc.scalar.copy(sampT[:, s * B:(s + 1) * B], ps[:])
                else:
                    nc.vector.tensor_copy(sampT[:, s * B:(s + 1) * B], ps[:])
            sqT = sp.tile([D, SB], fp)
            nc.scalar.activation(sqT[:], sampT[:], Act.Square)

            if stage < 70:
                return
            # ================= class shard stats =================
            msq = scrp.tile([CLOC, D], fp, tag="scCD")
            ssm = scrp.tile([CLOC, 1], fp, tag="scC")
            nc.scalar.activation(msq[:], wmu[:], Act.Square, accum_out=ssm[:])
            nm = scrp.tile([CLOC, 1], fp, tag="scC")
            nc.scalar.activation(nm[:], ssm[:], Act.Sqrt)
            nc.vector.tensor_scalar_max(nm[:], nm[:], 1e-12)
            rnm = scrp.tile([CLOC, 1], fp, tag="scC")
            nc.vector.reciprocal(rnm[:], nm[:])
            kap = sp.tile([CLOC, D], fp)
            nc.vector.tensor_scalar_max(kap[:], wk[:], 0.1)
            scm = sp.tile([CLOC, D], fp)
            nc.vector.scalar_tensor_tensor(scm[:], wmu[:], rnm[:], kap[:],
                                           op0=Alu.mult, op1=Alu.mult)
            csq = scrp.tile([CLOC, D], fp, tag="scCD")
            ssc = sp.tile([CLOC, 1], fp)
            nc.scalar.activation(csq[:], scm[:], Act.Square, accum_out=ssc[:])
            Pp = sp.tile([CLOC, D], fp)
            nc.vector.tensor_mul(Pp[:], kap[:], scm[:])
            Qq = sp.tile([CLOC, D], fp)
            nc.vector.tensor_mul(Qq[:], kap[:], kap[:])
            lkt = scrp.tile([CLOC, D], fp, tag="scCD")
            slk = sp.tile([CLOC, 1], fp)
            nc.scalar.activation(lkt[:], kap[:], Act.Ln, accum_out=slk[:])
            G = sp.tile([CLOC, 1], fp)
            nc.vector.tensor_scalar_add(G[:], ssc[:], 63.0 * 63.0)
            eta = sp.tile([CLOC, 1], fp)
            nc.scalar.activation(eta[:], G[:], Act.Sqrt)
            etap = scrp.tile([CLOC, 1], fp, tag="scC")
            nc.vector.tensor_scalar_add(etap[:], eta[:], 63.0)
            l63 = scrp.tile([CLOC, 1], fp, tag="scC")
            nc.scalar.activation(l63[:], etap[:], Act.Ln)
            lnG = scrp.tile([CLOC, 1], fp, tag="scC")
            nc.scalar.activation(lnG[:], G[:], Act.Ln)
            lnssc = scrp.tile([CLOC, 1], fp, tag="scC")
            nc.scalar.activation(lnssc[:], ssc[:], Act.Ln)
            c1 = scrp.tile([CLOC, 1], fp, tag="scC")
            nc.vector.scalar_tensor_tensor(c1[:], l63[:], 63.0, eta[:],
                                           op0=Alu.mult, op1=Alu.subtract)
            c2 = scrp.tile([CLOC, 1], fp, tag="scC")
            nc.vector.scalar_tensor_tensor(c2[:], lnssc[:], -0.5, slk[:],
                                           op0=Alu.mult, op1=Alu.add)
            nc.vector.tensor_add(c1[:], c1[:], c2[:])
            cst = sp.tile([CLOC, 1], fp)
            nc.vector.scalar_tensor_tensor(cst[:], lnG[:], 0.25, c1[:],
                                           op0=Alu.mult, op1=Alu.add)
            nc.vector.tensor_scalar_add(cst[:], cst[:], -63.5 * LN2PI)

            # transposes: PpT/QqT [D, CLOC], const row, const broadcast
            PpT = sp.tile([D, CLOC], fp)
            ps = ptp.tile([128, CLOC], fp, tag="tp")
            nc.tensor.transpose(ps[:], Pp[:], ident[0:CLOC, 0:CLOC])
            nc.scalar.copy(PpT[:], ps[:])
            QqT = sp.tile([D, CLOC], fp)
            ps = ptp.tile([128, CLOC], fp, tag="tp")
            nc.tensor.transpose(ps[:], Qq[:], ident[0:CLOC, 0:CLOC])
            nc.vector.tensor_copy(QqT[:], ps[:])
            cstT = sp.tile([1, CLOC], fp)
            ps = ptp.tile([1, CLOC], fp, tag="tp")
            nc.tensor.transpose(ps[:], cst[:], ident[0:CLOC, 0:CLOC])
            nc.scalar.copy(cstT[:], ps[:])
            cstB = sp.tile([128, CLOC], fp)
            ps = ptp.tile([128, CLOC], fp, tag="tp")
            nc.tensor.matmul(ps[:], ones[:], cstT[:], start=True, stop=True)
            nc.scalar.copy(cstB[:], ps[:])

            if stage < 80:
                return
            # ================= main matmuls + epilogue =================
            for mc in range(4):
                pn = pnp.tile([128, CLOC], fp, tag="pn")
                nc.tensor.matmul(pn[:], sampT[:, mc * 128:(mc + 1) * 128], PpT[:],
                                 start=True, stop=True)
                pd = pdp.tile([128, CLOC], fp, tag="pd")
                nc.tensor.matmul(pd[:], sqT[:, mc * 128:(mc + 1) * 128], QqT[:],
                                 start=True, stop=True)
                sd = scrp.tile([128, CLOC], fp, tag="ep")
                nc.scalar.activation(sd[:], pd[:], Act.Sqrt)
                rd = scrp.tile([128, CLOC], fp, tag="ep")
                nc.vector.reciprocal(rd[:], sd[:])
                m1 = scrp.tile([128, CLOC], fp, tag="ep")
                nc.vector.tensor_mul(m1[:], pn[:], rd[:])
                o = scrp.tile([128, CLOC], fp, tag="out")
                nc.vector.tensor_add(o[:], m1[:], cstB[:])
                nc.sync.dma_start(d_out[mc * 128:(mc + 1) * 128, :], o[:])

    with tile.TileContext(nc) as tc:
        _emit(tc)
    nc.finalize()
    return nc


def _get_nc():
    if "nc" not in _cache:
        _cache["nc"] = build_nc()
    return _cache["nc"]


def make_in_maps(inputs):
    eps_b, logu_b, v_b, ident = _host_constants()
    f32 = np.float32
    com = {
        "features": np.ascontiguousarray(inputs["features"], f32),
        "W0": np.ascontiguousarray(inputs["W0"], f32),
        "b0": np.ascontiguousarray(inputs["b0"], f32),
        "W1": np.ascontiguousarray(inputs["W1"], f32),
        "b1": np.ascontiguousarray(inputs["b1"], f32),
        "W2": np.ascontiguousarray(inputs["W2"], f32),
        "b2": np.ascontiguousarray(inputs["b2"], f32),
        "eps": eps_b, "logu": logu_b, "vt": v_b, "ident": ident,
    }
    wmu = np.ascontiguousarray(inputs["W_mu"], f32)
    wk = np.ascontiguousarray(inputs["W_kappa"], f32)
    in_maps = []
    for i in range(NCORES):
        m = dict(com)
        m["wmu"] = np.ascontiguousarray(wmu[i * CLOC:(i + 1) * CLOC])
        m["wk"] = np.ascontiguousarray(wk[i * CLOC:(i + 1) * CLOC])
        in_maps.append(m)
    return in_maps


def kernel(**inputs):
    from concourse.bass_utils import run_bass_kernel_spmd

    nc = _get_nc()
    in_maps = make_in_maps(inputs)
    res = run_bass_kernel_spmd(nc, in_maps, list(range(NCORES)))
    parts = [res.results[i]["out"].reshape(S, B, CLOC) for i in range(NCORES)]
    return np.ascontiguousarray(np.concatenate(parts, axis=2).astype(np.float32))

